# revision 1
# baseline (speedup 1.0000x reference)
"""CondConv2d Trainium2 kernel.

Problem: per-sample 3x3 'same' conv, B=16, CIN=COUT=32, H=W=256, with
per-sample weights mixed from 8 experts by routing weights.

Strategy:
- Host: tiny routing matmuls ([16,8]@[8,9216] and [16,8]@[8,32]) produce
  per-sample conv weights + bias. Inputs zero-padded to a [258, 260] layout
  so the device kernel needs no edge handling.
- Device (8 NeuronCores, 2 samples each): conv = 3 PSUM-accumulated fp32r
  matmuls per output tile (one per kernel row, K = 3*CIN = 96 via 3
  kw-shifted input replicas stacked in SBUF partitions; bias folded in as a
  97th contraction row against a ones-row). fp32r streams at 1 cycle/row
  (4x faster than fp32) with ~1e-4 relative error.
- Replicas: strip DMA loads partitions 0-31 once from HBM; two on-chip DVE
  copies produce the kw=1,2 shifts at partitions 32-95. Ones row at
  partition 96.
- PSUM: all matmul outputs at partitions 0-31 (fp32r requires dst partition
  0). Chunks of 8 output rows = [32, 2048] = 4 PSUM banks, double buffered.
  ScalarE evacuates PSUM->SBUF, then DMA to HBM.
"""

import numpy as np

B, CIN, H, W = 16, 32, 256, 256
COUT, KH, KW = 32, 3, 3
NCORES = 8
BPC = B // NCORES  # samples per core

RP = 260  # padded row pitch
PADROWS = H + 2  # 258
STRIP_OUT = 32  # output rows per strip
STRIP_ROWS = STRIP_OUT + 2  # 34
SFREE = STRIP_ROWS * RP  # 8840
NSTRIPS = H // STRIP_OUT  # 8
CHUNK_OUT = 8  # output rows per PSUM chunk
NCHUNK = STRIP_OUT // CHUNK_OUT  # 4
NT = CHUNK_OUT // 2  # matmul N-tiles per chunk (N=512 = 2 rows)
PSUM_BUFS = 2

_cache = {}


def _build():
    import concourse.bacc as bacc
    import concourse.mybir as mybir
    from concourse.tile import TileContext

    F32R = mybir.dt.float32r
    F32 = mybir.dt.float32

    nc = bacc.Bacc(name="condconv")
    x_d = nc.dram_tensor("xp", [BPC, CIN, PADROWS, RP], F32R, kind="ExternalInput")
    w_d = nc.dram_tensor("wt", [BPC, 97, KH * COUT], F32R, kind="ExternalInput")
    o_d = nc.dram_tensor("ones", [1, SFREE], F32R, kind="ExternalInput")
    y_d = nc.dram_tensor("y", [BPC, COUT, H, W], F32, kind="ExternalOutput")

    with TileContext(nc) as tc:
        with (
            tc.tile_pool(name="strip", bufs=4) as strip_pool,
            tc.tile_pool(name="wtp", bufs=2) as wt_pool,
            tc.tile_pool(name="stage", bufs=6) as stage_pool,
            tc.tile_pool(name="psum", bufs=PSUM_BUFS, space="PSUM") as psum_pool,
        ):
            def load_strip(b, s, first):
                r0 = s * STRIP_OUT
                strip = strip_pool.tile([128, SFREE], F32R, name="strip", tag="strip")
                s3 = strip.rearrange("p (y u) -> p y u", y=STRIP_ROWS)
                nc.sync.dma_start(
                    out=s3[0:CIN], in_=x_d[b, :, r0 : r0 + STRIP_ROWS, :]
                )
                # replica2 (kw=2): on-chip shift-by-2 copy of replica0 —
                # emitted before replica1's DMA so DVE starts as soon as
                # replica0 lands (it is the strip's longest-latency writer)
                nc.vector.tensor_copy(
                    out=strip[64:96, 0 : SFREE - 2], in_=strip[0:32, 2:SFREE]
                )
                # replica1 (kw=1 shift) loaded again from HBM: same rows,
                # written one column left (data at u=2+w instead of 3+w)
                nc.sync.dma_start(
                    out=s3[32:64, :, 2:258],
                    in_=x_d[b, :, r0 : r0 + STRIP_ROWS, 3:259],
                )
                nc.sync.dma_start(out=strip[96:97, :], in_=o_d[:])
                return s3

            def emit_chunks(b, s, s3, wt, last):
                r0 = s * STRIP_OUT
                for q in range(NCHUNK):
                    ps = psum_pool.tile([32, NT * 512], F32)
                    # group-major: all g0 matmuls (replica0+ones only), then
                    # g1, g2 — lets PE start before later replicas land +
                    # amortizes weight loads across NT consecutive matmuls
                    for g in range(KH):
                        k = 97 if g == 0 else 96
                        for nt in range(NT):
                            t = q * CHUNK_OUT + nt * 2
                            nc.tensor.matmul(
                                ps[:, nt * 512 : (nt + 1) * 512],
                                wt[0:k, g * COUT : (g + 1) * COUT],
                                s3[0:k, t + g : t + g + 2, 2 : 2 + W],
                                start=(g == 0),
                                stop=(g == KH - 1),
                            )
                    stage = stage_pool.tile([32, NT * 512], F32)
                    nc.scalar.copy(out=stage, in_=ps)
                    # dispatch output DMA from the idle GpSimd engine (SWDGE)
                    # so neither ACT's evac stream nor SP's input queues wait
                    # behind output FIFOs
                    lo = r0 + q * CHUNK_OUT
                    nc.gpsimd.dma_start(
                        out=y_d[b, :, lo : lo + CHUNK_OUT, :],
                        in_=stage.rearrange("p (r w) -> p r w", r=CHUNK_OUT),
                    )

            # software pipeline: emit strip s+1's loads/copies BEFORE strip
            # s's matmul+evac work so in-order engines prefetch one strip ahead
            order = [(b, s) for b in range(BPC) for s in range(NSTRIPS)]
            wts = {}
            pending = None
            for b, s in order:
                if s == 0:
                    wt = wt_pool.tile([97, KH * COUT], F32R, name="wt", tag="wt")
                    nc.sync.dma_start(out=wt, in_=w_d[b])
                    wts[b] = wt
                s3 = load_strip(b, s, first=(b == 0 and s == 0))
                if pending is not None:
                    pb, ps_, ps3 = pending
                    emit_chunks(pb, ps_, ps3, wts[pb], last=False)
                pending = (b, s, s3)
            pb, ps_, ps3 = pending
            emit_chunks(pb, ps_, ps3, wts[pb], last=True)
    nc.compile()
    return nc


def kernel(x, routing_weights, expert_weight, expert_bias):
    from concourse import bass_utils

    x = np.ascontiguousarray(x, dtype=np.float32)
    routing_weights = np.asarray(routing_weights, dtype=np.float32)
    expert_weight = np.asarray(expert_weight, dtype=np.float32)
    expert_bias = np.asarray(expert_bias, dtype=np.float32)

    # Host: routing mix (trivial flops) + weight/input repacking.
    w_all = routing_weights @ expert_weight  # [B, COUT*CIN*KH*KW]
    bias = routing_weights @ expert_bias  # [B, COUT]
    w4 = w_all.reshape(B, COUT, CIN, KH, KW)
    # lhsT[b, c*CIN+ci, r, co] = w4[b, co, ci, r, c]; row 96 = bias (group 0)
    lhsT = np.zeros((B, 97, KH, COUT), np.float32)
    lhsT[:, : KW * CIN] = w4.transpose(0, 4, 2, 3, 1).reshape(B, KW * CIN, KH, COUT)
    lhsT[:, 96, 0, :] = bias
    wt_h = np.ascontiguousarray(lhsT.reshape(B, 97, KH * COUT))

    xp = np.zeros((B, CIN, PADROWS, RP), np.float32)
    xp[:, :, 1 : 1 + H, 3 : 3 + W] = x
    ones = np.ones((1, SFREE), np.float32)

    if "nc" not in _cache:
        _cache["nc"] = _build()
    nc = _cache["nc"]

    in_maps = [
        {
            "xp": np.ascontiguousarray(xp[c * BPC : (c + 1) * BPC]),
            "wt": wt_h[c * BPC : (c + 1) * BPC],
            "ones": ones,
        }
        for c in range(NCORES)
    ]
    import os

    trace = bool(int(os.environ.get("CONDCONV_TRACE", "0")))
    res = bass_utils.run_bass_kernel_spmd(
        nc, in_maps, core_ids=list(range(NCORES)), trace=trace
    )
    _cache["last_results"] = res
    y = np.concatenate([res.results[c]["y"] for c in range(NCORES)], axis=0)
    return y

